# revision 32
# baseline (speedup 1.0000x reference)
"""Trainium2 Bass kernel for AdjacencyMatchingLoss (8-core SPMD).

Math: adj_score[b,e] = P[b,i_e,:] @ A @ P[b,j_e,:]  with A = (d_hw==1).
Let W[i,j] = sum_e w_e * 1[i_e=i] * 1[j_e=j]   (weighted pair histogram)
and Gm = sum_b P_b A P_b^T scaled by -1/8 (sign + batch mean folded into
the A mask). Then the per-core partial numerator is <W, Gm>.

Structure (v6, HW-measured ~2.46us/rep vs 12.4us baseline):
- Host ships PT (P transposed to [q, b*l], fp8e4m3) + w (bf16) + Asc
  (the scaled mask, elementwise) + a f32 ones column in one pm tensor.
  Z = matmul(lhsT=Asc, rhs=PT) gives (P_b A)^T slabs and
  G_b = matmul(lhsT=Z_b, rhs=PT_b) = P_b A P_b^T — no on-device
  transposes. Gm accumulates over b in one PSUM group.
- Edges are bucketed on the host by (i//32, j//16) into 32 groups and
  dealt round-robin across the 8 cores (any partition of the edge set
  is valid — the loss sums over all edges). Each group has a FIXED
  2-chunk (256-slot) per-core allocation, so the instruction schedule
  is input-independent (group-per-core count <=256 holds at ~12 sigma
  for iid-uniform edges; asserted). One-hots are then 32+16 wide
  (48B/edge vs 256B/edge ungrouped): 393KB of one-hot DMA per core
  instead of 1.6MB, and the W matmuls' stationary operand is 32 cols,
  cutting the (cost-model-invisible) per-chunk LDWEIGHTS ~4x.
- W chain: 56 fp8 matmuls into [32,16] PSUM blocks at [32*bi, 16*bj]
  via tile_position=(0,32*bi), bi cycling fastest (PE subarray
  concurrency). The bi=0 groups fuse their 2 chunks into ONE DoubleRow
  K=256 instr each (dst partition base 0 is the only DR-legal
  placement: s3d3_mm_valid_dst_partition); bi>0 groups emit 2 plain
  instrs. ONE accumulation group for all blocks: start=True clears
  has_written for the whole bank ON THE PARTITIONS THE MATMUL WRITES,
  so exactly the first instr of each col group carries start=True;
  later instrs overwrite-init their never-written cells and accumulate
  on the second pass.
- Queue discipline: SP/ACT queues carry ONLY input DMA issues (pm on
  SP, the single oh piece on ACT — one per-dma_start fixed cost per
  ring); all copies/reductions live on DVE; the
  output goes via the Pool SWDGE queue. In the 12.4us baseline the
  out-DMA sat at the end of the sync ring, so (FIFO per engine) rep
  n+1's input DMAs could not issue until rep n's compute drained.
- PE order per rep: Z (needs only pm) -> W (oh landed during previous
  reps) -> G, so the Zsb PSUM->SBUF copy overlaps the W matmuls.
- Tail: DVE multiply (W PSUM x Gm SBUF) + row-reduce to prt[:,0:1]
  (wsum reduced into prt[:,1:2] early), then a tiny f32 matmul
  (lhsT=prt, rhs=ones) folds 128 partitions to a [2,1] output so the
  per-rep SWDGE out-DMA carries 2 descriptors instead of 144.
- SBUF tiles are multi-buffered (SBUF_BUFS) so the input stream runs
  SBUF_BUFS-1 reps ahead of compute; bufs=3 vs 2 measured 4.8us->2.6us
  (the DMA subsystem here has ~us-scale per-dma_start fixed costs, so
  lookahead depth, not bytes, decides). PSUM stays single-buffered
  (all-PSUM bufs=2 was HW-incorrect at high rep counts despite passing
  CoreSim).

HW pitfalls hit while building this (axon-tunneled trn2, walrus
codegen): nc.vector.tensor_tensor_reduce wedges the exec unit
(NRT_EXEC_UNIT_UNRECOVERABLE, ~10min recovery) — use separate
tensor_tensor + tensor_reduce; per-block start=True clobbers sibling
PSUM blocks in the same bank (bank-wide has_written clear, but only on
the written partitions); DoubleRow + nonzero dst partition base fails
the ISA check only at walrus NEFF build (nc.compile() does not run it).

The w values ride inside OhIW in fp8 (~2% per-edge rounding, random
sign, averages out over 50k edges); P in fp8 (errors average in the
q-contraction). Overall rel err HW-measured ~2.3e-5.
"""

import os
import sys

import numpy as np

for _p in ("/opt/trn_rl_repo",):
    if os.path.isdir(_p) and _p not in sys.path:
        sys.path.insert(0, _p)

B, NL, NQ, E = 8, 128, 128, 50000
NCORES = 8

GROUPED = True
USE_DR = True

if GROUPED:
    WI, WJ = 32, 16              # one-hot widths (i-block, j-block)
    GI, GJ = NQ // WI, NQ // WJ  # 4 x 8 = 32 groups
    NGRP = GI * GJ
    CG = 2                       # chunks per group (256-slot capacity)
    CHUNKS = NGRP * CG           # 64
    SLOTS = CHUNKS * 128         # 8192 edge slots per core
    CHUNK_B = WI + WJ            # 48 B per chunk per partition
    PIECE_CHUNKS = [(0, CHUNKS)]
else:
    WI, WJ = 128, 128
    CHUNKS = ((E // NCORES) + 127) // 128   # 49
    SLOTS = CHUNKS * 128
    CHUNK_B = WI + WJ
    PIECE_CHUNKS = [(0, 25), (25, CHUNKS)]

OH_W = CHUNKS * CHUNK_B          # oh bytes per partition
W_B = 2 * CHUNKS                 # bf16 w slab bytes per partition

# pm_in packs PT + meta into ONE byte tensor [128, PM_W] (single DMA):
#   [0:1024)B PT fp8 | w bf16 | Asc fp8 | ones f32
PT_B = 1024
PM_W = PT_B + W_B + 128 + 4
SBUF_BUFS = 3
PSUM_BUFS = 1
DIAG = None  # None | 'dma' (DMAs only) | 'nogm' (skip Gm) | 'now' (skip W)

_BUILT = None


def _piece_off():
    offs, off = [], 0
    for c0, c1 in PIECE_CHUNKS:
        offs.append(off)
        off += (c1 - c0) * CHUNK_B
    return offs


def _iw_ap(c):
    for (c0, c1), po in zip(PIECE_CHUNKS, _piece_off()):
        if c0 <= c < c1:
            return po + (c - c0) * WI
    raise AssertionError(c)


def _j_ap(c):
    for (c0, c1), po in zip(PIECE_CHUNKS, _piece_off()):
        if c0 <= c < c1:
            return po + (c1 - c0) * WI + (c - c0) * WJ
    raise AssertionError(c)


def _emit_body(nc, sp, pp, tensors):
    import concourse.mybir as mybir

    f32 = mybir.dt.float32
    bf16 = mybir.dt.bfloat16
    i8 = mybir.dt.int8
    fp8 = mybir.dt.float8e4
    MUL = mybir.AluOpType.mult
    ADD = mybir.AluOpType.add
    DRM = mybir.MatmulPerfMode.DoubleRow
    in_d, o_d = tensors

    inp = sp.tile([128, PM_W + OH_W], i8)
    pm = inp
    oh = inp[:, PM_W : PM_W + OH_W].bitcast(fp8)
    Zsb = sp.tile([128, B * NL], bf16)
    GmS = sp.tile([128, NL], bf16)
    scr = sp.tile([128, NL], f32)
    prt = sp.tile([128, 2], f32)
    osb = sp.tile([2, 1], f32)

    Zps = pp.tile([128, B * NL], f32)
    Gps = pp.tile([128, NL], f32)
    Wps = pp.tile([128, NL], f32)
    Ops = pp.tile([2, 1], f32)

    # ---- exactly TWO byte-balanced input dma_starts, one per HWDGE
    # ring (each dma_start costs ~1us fixed on its ring on top of
    # bytes, so 2 balanced transfers beat pm+oh as 3) ----
    SPLIT = (PM_W + OH_W) // 2
    nc.sync.dma_start(out=inp[:, 0:SPLIT], in_=in_d.ap()[:, 0:SPLIT])
    nc.scalar.dma_start(
        out=inp[:, SPLIT : PM_W + OH_W],
        in_=in_d.ap()[:, SPLIT : PM_W + OH_W],
    )

    # views into pm (byte offsets)
    PT = pm[:, 0:PT_B].bitcast(fp8)                        # [128, 1024]
    wT = pm[:, PT_B : PT_B + W_B].bitcast(bf16)            # [128, CHUNKS]
    Asc = pm[:, PT_B + W_B : PT_B + W_B + 128].bitcast(fp8)
    ones = pm[:, PT_B + W_B + 128 : PT_B + W_B + 132].bitcast(f32)

    nc.vector.tensor_reduce(
        out=prt[:, 1:2], in_=wT, axis=mybir.AxisListType.X, op=ADD
    )

    def emit_out():
        # fold the 128 per-partition partials to [2,1] on the PE so the
        # per-rep SWDGE out-DMA carries 2 descriptors instead of 144
        nc.tensor.matmul(
            Ops[:], lhsT=prt[:], rhs=ones, start=True, stop=True
        )
        nc.vector.tensor_copy(out=osb[:], in_=Ops[:])
        nc.gpsimd.dma_start(out=o_d.ap(), in_=osb[:])

    if DIAG == "dma":
        nc.vector.tensor_reduce(
            out=prt[:, 0:1], in_=oh[:, 0:128].bitcast(i8),
            axis=mybir.AxisListType.X, op=ADD,
        )
        emit_out()
        return

    # ---- Z = (P_b A)^T slabs: two 512-wide matmuls, Asc stationary ----
    if DIAG != "nogm":
        for h in range(2):
            sl = slice(h * 512, (h + 1) * 512)
            nc.tensor.matmul(
                Zps[:, sl], lhsT=Asc, rhs=PT[:, sl], start=True, stop=True
            )

    # ---- W accumulation from the one-hot stream ----
    if DIAG != "now":
        if GROUPED:
            # ONE accumulation group across all 32 disjoint [32,WJ]
            # blocks: start=True clears has_written for the WHOLE bank,
            # so per-block start flags would clobber sibling blocks.
            # With a single group, each block's first write lands on
            # cleared has_written (overwrite-init) and its second
            # accumulates. bi cycles fastest so consecutive instrs hit
            # different PE column groups (subarray concurrency).
            # instruction list: bi==0 groups fuse their 2 chunks into
            # ONE DoubleRow K=256 instr (dst partition base 0 is the
            # only DR-legal placement: s3d3_mm_valid_dst_partition);
            # bi>0 groups emit CG plain instrs. 56 instrs vs 64.
            instrs = []
            for r in range(CG):
                for bj in range(GJ):
                    for bi in range(GI):
                        if USE_DR and bi == 0 and r > 0:
                            continue
                        c = (bi * GJ + bj) * CG + r
                        instrs.append((bi, bj, c, USE_DR and bi == 0))
            started = set()
            for idx, (bi, bj, c, dr) in enumerate(instrs):
                blk = Wps[32 * bi : 32 * bi + 32,
                          WJ * bj : WJ * bj + WJ]
                # the bank-bit clear is per-partition: the first instr
                # of EACH col group (disjoint partition ranges) must
                # carry start=True
                kw = dict(
                    start=(bi not in started),
                    stop=(idx == len(instrs) - 1),
                    tile_position=(0, 32 * bi),
                    skip_group_check=True,
                )
                started.add(bi)
                if dr:
                    lt = oh[:, _iw_ap(c) : _iw_ap(c) + 2 * WI]
                    rt = oh[:, _j_ap(c) : _j_ap(c) + 2 * WJ]
                    nc.tensor.matmul(
                        blk,
                        lhsT=lt.rearrange("p (two m) -> p two m", two=2),
                        rhs=rt.rearrange("p (two m) -> p two m", two=2),
                        perf_mode=DRM, **kw,
                    )
                else:
                    nc.tensor.matmul(
                        blk,
                        lhsT=oh[:, _iw_ap(c) : _iw_ap(c) + WI],
                        rhs=oh[:, _j_ap(c) : _j_ap(c) + WJ],
                        **kw,
                    )
        else:
            c = 0
            while c < CHUNKS:
                if USE_DR and c + 1 < CHUNKS and _iw_ap(c + 1) == _iw_ap(c) + 128:
                    two = lambda a: oh[:, a : a + 256].rearrange(
                        "p (two m) -> p two m", two=2
                    )
                    nc.tensor.matmul(
                        Wps[:], lhsT=two(_iw_ap(c)), rhs=two(_j_ap(c)),
                        start=(c == 0), stop=(c + 2 >= CHUNKS),
                        perf_mode=DRM,
                    )
                    c += 2
                else:
                    nc.tensor.matmul(
                        Wps[:], lhsT=oh[:, _iw_ap(c) : _iw_ap(c) + 128],
                        rhs=oh[:, _j_ap(c) : _j_ap(c) + 128],
                        start=(c == 0), stop=(c + 1 >= CHUNKS),
                    )
                    c += 1

    # ---- Gm = sum_b P_b Asc P_b^T (second stage; Zsb copy overlaps W) --
    if DIAG != "nogm":
        nc.vector.tensor_copy(out=Zsb[:], in_=Zps[:])
        for b in range(B):
            sl = slice(b * 128, (b + 1) * 128)
            nc.tensor.matmul(
                Gps[:], lhsT=Zsb[:, sl], rhs=PT[:, sl],
                start=(b == 0), stop=(b == B - 1),
            )
        nc.vector.tensor_copy(out=GmS[:], in_=Gps[:])

    # ---- tail: partial[p] = sum_j W[p,j] * Gm[p,j] ----
    if DIAG == "now":
        in0, in1 = Gps[:], GmS[:]
    else:
        in0 = Wps[:]
        in1 = Asc if DIAG == "nogm" else GmS[:]
    nc.vector.tensor_tensor(out=scr[:], in0=in0, in1=in1, op=MUL)
    nc.vector.tensor_reduce(
        out=prt[:, 0:1], in_=scr[:], axis=mybir.AxisListType.X, op=ADD
    )
    emit_out()


def _build(reps=1):
    import concourse.bacc as bacc
    import concourse.mybir as mybir
    import concourse.tile as tile

    f32 = mybir.dt.float32
    fp8 = mybir.dt.float8e4

    nc = bacc.Bacc("TRN2", target_bir_lowering=False, debug=False, num_devices=NCORES)

    in_d = nc.dram_tensor(
        "in_all", [128, PM_W + OH_W], mybir.dt.int8, kind="ExternalInput"
    )
    o_d = nc.dram_tensor("out", [2, 1], f32, kind="ExternalOutput")

    with tile.TileContext(nc) as tc:
        with (
            tc.tile_pool(name="sbuf", bufs=SBUF_BUFS) as sp,
            tc.tile_pool(name="psum", bufs=PSUM_BUFS, space="PSUM") as pp,
        ):
            for _ in range(reps):
                _emit_body(nc, sp, pp, (in_d, o_d))

    nc.compile()
    return nc


def _get_built():
    global _BUILT
    if _BUILT is None:
        _BUILT = _build()
    return _BUILT


def _shard_inputs(P, d_hw, circuit_edge_pairs, circuit_edge_weights):
    import ml_dtypes

    bf16 = ml_dtypes.bfloat16
    fp8 = ml_dtypes.float8_e4m3

    P = np.asarray(P, dtype=np.float32)
    d = np.asarray(d_hw, dtype=np.int32)
    pairs = np.asarray(circuit_edge_pairs).astype(np.int64, copy=False)
    w = np.asarray(circuit_edge_weights, dtype=np.float32)

    # PT[q, b*128 + l] = P[b, l, q]  (replicated to all cores)
    PT = np.ascontiguousarray(P.transpose(2, 0, 1).reshape(128, B * NL))

    i_idx = pairs[:, 0].astype(np.int64)
    j_idx = pairs[:, 1].astype(np.int64)

    ohiw = np.zeros((NCORES, 128, CHUNKS * WI), dtype=fp8)
    ohj = np.zeros((NCORES, 128, CHUNKS * WJ), dtype=fp8)
    wslab = np.zeros((NCORES, 128, CHUNKS), dtype=np.float32)

    if GROUPED:
        # bucket edges by (i//WI, j//WJ), deal each bucket round-robin
        # across cores; group g owns slots [g*CG*128, (g+1)*CG*128).
        g_e = (i_idx // WI) * GJ + (j_idx // WJ)
        order = np.argsort(g_e, kind="stable")
        gs = g_e[order]
        counts = np.bincount(gs, minlength=NGRP)
        gstart = np.zeros(NGRP, dtype=np.int64)
        gstart[1:] = np.cumsum(counts)[:-1]
        r = np.arange(E) - gstart[gs]
        core = r % NCORES
        k = r // NCORES
        assert int(k.max()) < CG * 128, "group-core bucket overflow"
        slot = gs * (CG * 128) + k
        chunk = slot // 128
        p = slot % 128
        io, jo, wo = i_idx[order], j_idx[order], w[order]
        ohiw[core, p, chunk * WI + (io - (gs // GJ) * WI)] = wo.astype(fp8)
        ohj[core, p, chunk * WJ + (jo - (gs % GJ) * WJ)] = fp8(1.0)
        wslab[core, p, chunk] = wo
    else:
        ESH = E // NCORES
        EPAD = CHUNKS * 128
        k = np.arange(EPAD)
        cc, p = k // 128, k % 128
        ip = np.zeros((NCORES, EPAD), dtype=np.int64)
        jp = np.zeros((NCORES, EPAD), dtype=np.int64)
        wp = np.zeros((NCORES, EPAD), dtype=np.float32)
        ip[:, :ESH] = i_idx.reshape(NCORES, ESH)
        jp[:, :ESH] = j_idx.reshape(NCORES, ESH)
        wp[:, :ESH] = w.reshape(NCORES, ESH)
        core = np.repeat(np.arange(NCORES), EPAD).reshape(NCORES, EPAD)
        pb = np.broadcast_to(p, (NCORES, EPAD))
        ohiw[core, pb, cc * 128 + ip] = wp.astype(fp8)
        ohj[core, pb, cc * 128 + jp] = fp8(1.0)
        wslab[core.reshape(-1), pb.reshape(-1), np.broadcast_to(cc, (NCORES, EPAD)).reshape(-1)] = 0  # noqa
        wslab[:] = 0
        wslab[core, pb, np.broadcast_to(cc, (NCORES, EPAD))] = wp

    # fuse into piece layout [IW_piece | J_piece | ...]
    oh = np.zeros((NCORES, 128, OH_W), dtype=fp8)
    off = 0
    for c0, c1 in PIECE_CHUNKS:
        niw = (c1 - c0) * WI
        nj = (c1 - c0) * WJ
        oh[:, :, off : off + niw] = ohiw[:, :, c0 * WI : c1 * WI]
        oh[:, :, off + niw : off + niw + nj] = ohj[:, :, c0 * WJ : c1 * WJ]
        off += niw + nj

    # pm: PT | w | Asc, byte layout
    asc = ((d == 1) * np.float32(-0.125)).astype(fp8)  # elementwise mask

    pm = np.zeros((NCORES, 128, PM_W), dtype=np.int8)
    pm[:, :, 0:PT_B] = PT.astype(fp8).view(np.int8)[None]
    pm[:, :, PT_B : PT_B + W_B] = np.ascontiguousarray(
        wslab.astype(bf16)
    ).view(np.int8)
    pm[:, :, PT_B + W_B : PT_B + W_B + 128] = asc.view(np.int8)[None]
    pm[:, :, PT_B + W_B + 128 : PT_B + W_B + 132] = (
        np.ones((128, 1), np.float32).view(np.int8)[None]
    )

    in_all = np.concatenate([pm, oh.view(np.int8)], axis=2)
    return [{"in_all": np.ascontiguousarray(in_all[i])} for i in range(NCORES)]


def _combine(results):
    parts = np.stack([np.asarray(results[i]["out"]) for i in range(NCORES)])
    numer = float(parts[:, 0, 0].astype(np.float64).sum())
    wsum = float(parts[:, 1, 0].astype(np.float64).sum())
    return np.float32(numer / max(wsum, 1e-8))


def make_runner(nc, n_cores=NCORES):
    """jit-once mirror of bass2jax.run_bass_via_pjrt's multi-core branch so
    repeated kernel() calls reuse the compiled NEFF."""
    import jax
    import concourse.mybir as mybir
    from concourse.bass2jax import (
        Mesh,
        PartitionSpec,
        _bass_exec_p,
        install_neuronx_cc_hook,
        partition_id_tensor,
        shard_map,
    )

    install_neuronx_cc_hook()
    partition_name = nc.partition_id_tensor.name if nc.partition_id_tensor else None

    in_names, out_names, out_avals, zero_outs = [], [], [], []
    for alloc in nc.m.functions[0].allocations:
        if not isinstance(alloc, mybir.MemoryLocationSet):
            continue
        name = alloc.memorylocations[0].name
        if alloc.kind == "ExternalInput":
            if name != partition_name:
                in_names.append(name)
        elif alloc.kind == "ExternalOutput":
            shape = tuple(alloc.tensor_shape)
            dtype = mybir.dt.np(alloc.dtype)
            out_names.append(name)
            out_avals.append(jax.core.ShapedArray(shape, dtype))
            zero_outs.append(np.zeros(shape, dtype))
    n_params = len(in_names)
    n_outs = len(out_avals)
    all_names = in_names + out_names
    if partition_name is not None:
        all_names = all_names + [partition_name]
    donate = tuple(range(n_params, n_params + n_outs))

    def _body(*args):
        operands = list(args)
        if partition_name is not None:
            operands.append(partition_id_tensor())
        outs = _bass_exec_p.bind(
            *operands,
            out_avals=tuple(out_avals),
            in_names=tuple(all_names),
            out_names=tuple(out_names),
            lowering_input_output_aliases=(),
            sim_require_finite=True,
            sim_require_nnan=True,
            nc=nc,
        )
        return tuple(outs)

    devices = jax.devices()[:n_cores]
    mesh = Mesh(np.asarray(devices), ("core",))
    sharded = jax.jit(
        shard_map(
            _body,
            mesh=mesh,
            in_specs=(PartitionSpec("core"),) * (n_params + n_outs),
            out_specs=(PartitionSpec("core"),) * n_outs,
            check_rep=False,
        ),
        donate_argnums=donate,
        keep_unused=True,
    )

    def prep(in_maps):
        concat_in = [
            np.concatenate([np.asarray(m[name]) for m in in_maps], axis=0)
            for name in in_names
        ]
        return [jax.device_put(a) for a in concat_in]

    def run_dev(dev_in):
        concat_zeros = [
            np.zeros((n_cores * z.shape[0], *z.shape[1:]), z.dtype)
            for z in zero_outs
        ]
        out_arrs = sharded(*dev_in, *concat_zeros)
        out_arrs = [np.asarray(a) for a in out_arrs]
        return [
            {
                name: out_arrs[i].reshape(n_cores, *out_avals[i].shape)[c]
                for i, name in enumerate(out_names)
            }
            for c in range(n_cores)
        ]

    def run(in_maps):
        return run_dev(prep(in_maps))

    run.prep = prep
    run.run_dev = run_dev
    return run


_RUNNER = None


def kernel(P, d_hw, circuit_edge_pairs, circuit_edge_weights, _want_results=False):
    global _RUNNER
    in_maps = _shard_inputs(P, d_hw, circuit_edge_pairs, circuit_edge_weights)
    try:
        if _RUNNER is None:
            _RUNNER = make_runner(_get_built())
        results = _RUNNER(in_maps)
        res = None
    except Exception:
        if _want_results:
            raise
        # fallback: the stock SPMD runner (recompiles per call, but robust)
        from concourse.bass_utils import run_bass_kernel_spmd

        res = run_bass_kernel_spmd(
            _get_built(), in_maps, core_ids=list(range(NCORES))
        )
        results = res.results
    out = _combine(results)
    if _want_results:
        return out, res
    return out


# revision 36
# speedup vs baseline: 1.1500x; 1.1500x over previous
"""Trainium2 Bass kernel for AdjacencyMatchingLoss (8-core SPMD).

Math: adj_score[b,e] = P[b,i_e,:] @ A @ P[b,j_e,:]  with A = (d_hw==1).
Let W[i,j] = sum_e w_e * 1[i_e=i] * 1[j_e=j]   (weighted pair histogram)
and Gm = sum_b P_b A P_b^T scaled by -1/8 (sign + batch mean folded into
the A mask). Then the per-core partial numerator is <W, Gm>.

Structure (v6, HW-measured ~2.46us/rep vs 12.4us baseline):
- Host ships PT (P transposed to [q, b*l], fp8e4m3) + w (bf16) + Asc
  (the scaled mask, elementwise) + a f32 ones column in one pm tensor.
  Z = matmul(lhsT=Asc, rhs=PT) gives (P_b A)^T slabs and
  G_b = matmul(lhsT=Z_b, rhs=PT_b) = P_b A P_b^T — no on-device
  transposes. Gm accumulates over b in one PSUM group.
- Edges are bucketed on the host by (i//32, j//16) into 32 groups and
  dealt round-robin across the 8 cores (any partition of the edge set
  is valid — the loss sums over all edges). Each group has a FIXED
  2-chunk (256-slot) per-core allocation, so the instruction schedule
  is input-independent (group-per-core count <=256 holds at ~12 sigma
  for iid-uniform edges; asserted). One-hots are then 32+16 wide
  (48B/edge vs 256B/edge ungrouped): 393KB of one-hot DMA per core
  instead of 1.6MB, and the W matmuls' stationary operand is 32 cols,
  cutting the (cost-model-invisible) per-chunk LDWEIGHTS ~4x.
- W chain: 56 fp8 matmuls into [32,16] PSUM blocks at [32*bi, 16*bj]
  via tile_position=(0,32*bi), bi cycling fastest (PE subarray
  concurrency). The bi=0 groups fuse their 2 chunks into ONE DoubleRow
  K=256 instr each (dst partition base 0 is the only DR-legal
  placement: s3d3_mm_valid_dst_partition rejects base 32 AND 64,
  HW-confirmed); bi>0 groups emit 2 plain
  instrs. ONE accumulation group for all blocks: start=True clears
  has_written for the whole bank ON THE PARTITIONS THE MATMUL WRITES,
  so exactly the first instr of each col group carries start=True;
  later instrs overwrite-init their never-written cells and accumulate
  on the second pass.
- Queue discipline: SP/ACT queues carry ONLY input DMA issues (pm on
  SP, the single oh piece on ACT — one per-dma_start fixed cost per
  ring); all copies/reductions live on DVE; the
  output goes via the Pool SWDGE queue. In the 12.4us baseline the
  out-DMA sat at the end of the sync ring, so (FIFO per engine) rep
  n+1's input DMAs could not issue until rep n's compute drained.
- PE order per rep: Z (needs only pm) -> W (oh landed during previous
  reps) -> G, so the Zsb PSUM->SBUF copy overlaps the W matmuls.
- Tail: DVE multiply (W PSUM x Gm SBUF) + row-reduce to prt[:,0:1]
  (wsum reduced into prt[:,1:2] early), then a tiny f32 matmul
  (lhsT=prt, rhs=ones) folds 128 partitions to a [2,1] output so the
  per-rep SWDGE out-DMA carries 2 descriptors instead of 144.
- SBUF tiles are multi-buffered (SBUF_BUFS) so the input stream runs
  SBUF_BUFS-1 reps ahead of compute; bufs=3 vs 2 measured 4.8us->2.6us
  (the DMA subsystem here has ~us-scale per-dma_start fixed costs, so
  lookahead depth, not bytes, decides). Fusing pm+oh into one DRAM
  tensor to cut dma_start count measured WORSE twice (3.25us vs 2.46us
  at bufs=3): one shared input tile gates the whole next transfer on
  the G-chain's last PT read, choking the lookahead — keep pm and oh
  as separate tiles/tensors. PSUM stays single-buffered
  (all-PSUM bufs=2 was HW-incorrect at high rep counts despite passing
  CoreSim).

HW pitfalls hit while building this (axon-tunneled trn2, walrus
codegen): nc.vector.tensor_tensor_reduce wedges the exec unit
(NRT_EXEC_UNIT_UNRECOVERABLE, ~10min recovery) — use separate
tensor_tensor + tensor_reduce; per-block start=True clobbers sibling
PSUM blocks in the same bank (bank-wide has_written clear, but only on
the written partitions); DoubleRow + nonzero dst partition base fails
the ISA check only at walrus NEFF build (nc.compile() does not run it).

The w values ride inside OhIW in fp8 (~2% per-edge rounding, random
sign, averages out over 50k edges); P in fp8 (errors average in the
q-contraction). Overall rel err HW-measured ~2.3e-5.
"""

import os
import sys

import numpy as np

for _p in ("/opt/trn_rl_repo",):
    if os.path.isdir(_p) and _p not in sys.path:
        sys.path.insert(0, _p)

B, NL, NQ, E = 8, 128, 128, 50000
NCORES = 8

GROUPED = True
USE_DR = True

if GROUPED:
    WI, WJ = 32, 16              # one-hot widths (i-block, j-block)
    GI, GJ = NQ // WI, NQ // WJ  # 4 x 8 = 32 groups
    NGRP = GI * GJ
    CG = 2                       # chunks per group (256-slot capacity)
    CHUNKS = NGRP * CG           # 64
    SLOTS = CHUNKS * 128         # 8192 edge slots per core
    CHUNK_B = WI + WJ            # 48 B per chunk per partition
    PIECE_CHUNKS = [(0, CHUNKS)]
else:
    WI, WJ = 128, 128
    CHUNKS = ((E // NCORES) + 127) // 128   # 49
    SLOTS = CHUNKS * 128
    CHUNK_B = WI + WJ
    PIECE_CHUNKS = [(0, 25), (25, CHUNKS)]

OH_W = CHUNKS * CHUNK_B          # oh bytes per partition
W_B = 2 * CHUNKS                 # bf16 w slab bytes per partition

# pm_in packs PT + meta into ONE byte tensor [128, PM_W] (single DMA):
#   [0:1024)B PT fp8 | w bf16 | Asc fp8 | ones f32
PT_B = 1024
PM_W = PT_B + W_B + 128 + 4
SBUF_BUFS = 3
PSUM_BUFS = 1
DIAG = None  # None | 'dma' (DMAs only) | 'nogm' (skip Gm) | 'now' (skip W)

_BUILT = None


def _piece_off():
    offs, off = [], 0
    for c0, c1 in PIECE_CHUNKS:
        offs.append(off)
        off += (c1 - c0) * CHUNK_B
    return offs


def _iw_ap(c):
    for (c0, c1), po in zip(PIECE_CHUNKS, _piece_off()):
        if c0 <= c < c1:
            return po + (c - c0) * WI
    raise AssertionError(c)


def _j_ap(c):
    for (c0, c1), po in zip(PIECE_CHUNKS, _piece_off()):
        if c0 <= c < c1:
            return po + (c1 - c0) * WI + (c - c0) * WJ
    raise AssertionError(c)


def _emit_body(nc, sp, pp, tensors):
    import concourse.mybir as mybir

    f32 = mybir.dt.float32
    bf16 = mybir.dt.bfloat16
    i8 = mybir.dt.int8
    fp8 = mybir.dt.float8e4
    MUL = mybir.AluOpType.mult
    ADD = mybir.AluOpType.add
    DRM = mybir.MatmulPerfMode.DoubleRow
    pm_d, oh_d, o_d = tensors

    pm = sp.tile([128, PM_W], i8)
    oh = sp.tile([128, OH_W], fp8)
    Zsb = sp.tile([128, B * NL], bf16)
    GmS = sp.tile([128, NL], bf16)
    scr = sp.tile([128, NL], f32)
    prt = sp.tile([128, 2], f32)
    osb = sp.tile([2, 1], f32)

    Zps = pp.tile([128, B * NL], f32)
    Gps = pp.tile([128, NL], f32)
    Wps = pp.tile([128, NL], f32)
    Ops = pp.tile([2, 1], f32)

    # ---- DMAs (emitted up front; SP and ACT rings carry ONLY dma issues
    # so rep n+1's transfers stream while rep n computes) ----
    nc.sync.dma_start(out=pm[:], in_=pm_d.ap())
    rings = [nc.scalar, nc.sync]
    for pi, ((c0, c1), po) in enumerate(zip(PIECE_CHUNKS, _piece_off())):
        sz = (c1 - c0) * CHUNK_B
        rings[pi % len(rings)].dma_start(
            out=oh[:, po : po + sz], in_=oh_d.ap()[:, po : po + sz]
        )

    # views into pm (byte offsets)
    PT = pm[:, 0:PT_B].bitcast(fp8)                        # [128, 1024]
    wT = pm[:, PT_B : PT_B + W_B].bitcast(bf16)            # [128, CHUNKS]
    Asc = pm[:, PT_B + W_B : PT_B + W_B + 128].bitcast(fp8)
    ones = pm[:, PT_B + W_B + 128 : PT_B + W_B + 132].bitcast(f32)

    nc.vector.tensor_reduce(
        out=prt[:, 1:2], in_=wT, axis=mybir.AxisListType.X, op=ADD
    )

    def emit_out():
        # fold the 128 per-partition partials to [2,1] on the PE so the
        # per-rep SWDGE out-DMA carries 2 descriptors instead of 144
        nc.tensor.matmul(
            Ops[:], lhsT=prt[:], rhs=ones, start=True, stop=True
        )
        nc.vector.tensor_copy(out=osb[:], in_=Ops[:])
        nc.gpsimd.dma_start(out=o_d.ap(), in_=osb[:])

    if DIAG == "dma":
        nc.vector.tensor_reduce(
            out=prt[:, 0:1], in_=oh[:, 0:128].bitcast(i8),
            axis=mybir.AxisListType.X, op=ADD,
        )
        emit_out()
        return

    # ---- Z = (P_b A)^T slabs: two 512-wide matmuls, Asc stationary ----
    if DIAG != "nogm":
        for h in range(2):
            sl = slice(h * 512, (h + 1) * 512)
            nc.tensor.matmul(
                Zps[:, sl], lhsT=Asc, rhs=PT[:, sl], start=True, stop=True
            )

    # ---- W accumulation from the one-hot stream ----
    if DIAG != "now":
        if GROUPED:
            # ONE accumulation group across all 32 disjoint [32,WJ]
            # blocks: start=True clears has_written for the WHOLE bank,
            # so per-block start flags would clobber sibling blocks.
            # With a single group, each block's first write lands on
            # cleared has_written (overwrite-init) and its second
            # accumulates. bi cycles fastest so consecutive instrs hit
            # different PE column groups (subarray concurrency).
            # instruction list: bi==0 groups fuse their 2 chunks into
            # ONE DoubleRow K=256 instr (dst partition base 0 is the
            # only DR-legal placement: s3d3_mm_valid_dst_partition);
            # bi>0 groups emit CG plain instrs. 56 instrs vs 64.
            instrs = []
            for r in range(CG):
                for bj in range(GJ):
                    for bi in range(GI):
                        if USE_DR and bi == 0 and r > 0:
                            continue
                        c = (bi * GJ + bj) * CG + r
                        instrs.append((bi, bj, c, USE_DR and bi == 0))
            started = set()
            for idx, (bi, bj, c, dr) in enumerate(instrs):
                blk = Wps[32 * bi : 32 * bi + 32,
                          WJ * bj : WJ * bj + WJ]
                # the bank-bit clear is per-partition: the first instr
                # of EACH col group (disjoint partition ranges) must
                # carry start=True
                kw = dict(
                    start=(bi not in started),
                    stop=(idx == len(instrs) - 1),
                    tile_position=(0, 32 * bi),
                    skip_group_check=True,
                )
                started.add(bi)
                if dr:
                    lt = oh[:, _iw_ap(c) : _iw_ap(c) + 2 * WI]
                    rt = oh[:, _j_ap(c) : _j_ap(c) + 2 * WJ]
                    nc.tensor.matmul(
                        blk,
                        lhsT=lt.rearrange("p (two m) -> p two m", two=2),
                        rhs=rt.rearrange("p (two m) -> p two m", two=2),
                        perf_mode=DRM, **kw,
                    )
                else:
                    nc.tensor.matmul(
                        blk,
                        lhsT=oh[:, _iw_ap(c) : _iw_ap(c) + WI],
                        rhs=oh[:, _j_ap(c) : _j_ap(c) + WJ],
                        **kw,
                    )
        else:
            c = 0
            while c < CHUNKS:
                if USE_DR and c + 1 < CHUNKS and _iw_ap(c + 1) == _iw_ap(c) + 128:
                    two = lambda a: oh[:, a : a + 256].rearrange(
                        "p (two m) -> p two m", two=2
                    )
                    nc.tensor.matmul(
                        Wps[:], lhsT=two(_iw_ap(c)), rhs=two(_j_ap(c)),
                        start=(c == 0), stop=(c + 2 >= CHUNKS),
                        perf_mode=DRM,
                    )
                    c += 2
                else:
                    nc.tensor.matmul(
                        Wps[:], lhsT=oh[:, _iw_ap(c) : _iw_ap(c) + 128],
                        rhs=oh[:, _j_ap(c) : _j_ap(c) + 128],
                        start=(c == 0), stop=(c + 1 >= CHUNKS),
                    )
                    c += 1

    # ---- Gm = sum_b P_b Asc P_b^T (second stage; Zsb copy overlaps W) --
    if DIAG != "nogm":
        nc.vector.tensor_copy(out=Zsb[:], in_=Zps[:])
        for b in range(B):
            sl = slice(b * 128, (b + 1) * 128)
            nc.tensor.matmul(
                Gps[:], lhsT=Zsb[:, sl], rhs=PT[:, sl],
                start=(b == 0), stop=(b == B - 1),
            )
        nc.vector.tensor_copy(out=GmS[:], in_=Gps[:])

    # ---- tail: partial[p] = sum_j W[p,j] * Gm[p,j] ----
    if DIAG == "now":
        in0, in1 = Gps[:], GmS[:]
    else:
        in0 = Wps[:]
        in1 = Asc if DIAG == "nogm" else GmS[:]
    nc.vector.tensor_tensor(out=scr[:], in0=in0, in1=in1, op=MUL)
    nc.vector.tensor_reduce(
        out=prt[:, 0:1], in_=scr[:], axis=mybir.AxisListType.X, op=ADD
    )
    emit_out()


def _build(reps=1):
    import concourse.bacc as bacc
    import concourse.mybir as mybir
    import concourse.tile as tile

    f32 = mybir.dt.float32
    fp8 = mybir.dt.float8e4

    nc = bacc.Bacc("TRN2", target_bir_lowering=False, debug=False, num_devices=NCORES)

    pm_d = nc.dram_tensor("pm_in", [128, PM_W], mybir.dt.int8, kind="ExternalInput")
    oh_d = nc.dram_tensor("oh_in", [128, OH_W], fp8, kind="ExternalInput")
    o_d = nc.dram_tensor("out", [2, 1], f32, kind="ExternalOutput")

    with tile.TileContext(nc) as tc:
        with (
            tc.tile_pool(name="sbuf", bufs=SBUF_BUFS) as sp,
            tc.tile_pool(name="psum", bufs=PSUM_BUFS, space="PSUM") as pp,
        ):
            for _ in range(reps):
                _emit_body(nc, sp, pp, (pm_d, oh_d, o_d))

    nc.compile()
    return nc


def _get_built():
    global _BUILT
    if _BUILT is None:
        _BUILT = _build()
    return _BUILT


def _shard_inputs(P, d_hw, circuit_edge_pairs, circuit_edge_weights):
    import ml_dtypes

    bf16 = ml_dtypes.bfloat16
    fp8 = ml_dtypes.float8_e4m3

    P = np.asarray(P, dtype=np.float32)
    d = np.asarray(d_hw, dtype=np.int32)
    pairs = np.asarray(circuit_edge_pairs).astype(np.int64, copy=False)
    w = np.asarray(circuit_edge_weights, dtype=np.float32)

    # PT[q, b*128 + l] = P[b, l, q]  (replicated to all cores)
    PT = np.ascontiguousarray(P.transpose(2, 0, 1).reshape(128, B * NL))

    i_idx = pairs[:, 0].astype(np.int64)
    j_idx = pairs[:, 1].astype(np.int64)

    ohiw = np.zeros((NCORES, 128, CHUNKS * WI), dtype=fp8)
    ohj = np.zeros((NCORES, 128, CHUNKS * WJ), dtype=fp8)
    wslab = np.zeros((NCORES, 128, CHUNKS), dtype=np.float32)

    if GROUPED:
        # bucket edges by (i//WI, j//WJ), deal each bucket round-robin
        # across cores; group g owns slots [g*CG*128, (g+1)*CG*128).
        g_e = (i_idx // WI) * GJ + (j_idx // WJ)
        order = np.argsort(g_e, kind="stable")
        gs = g_e[order]
        counts = np.bincount(gs, minlength=NGRP)
        gstart = np.zeros(NGRP, dtype=np.int64)
        gstart[1:] = np.cumsum(counts)[:-1]
        r = np.arange(E) - gstart[gs]
        core = r % NCORES
        k = r // NCORES
        assert int(k.max()) < CG * 128, "group-core bucket overflow"
        slot = gs * (CG * 128) + k
        chunk = slot // 128
        p = slot % 128
        io, jo, wo = i_idx[order], j_idx[order], w[order]
        ohiw[core, p, chunk * WI + (io - (gs // GJ) * WI)] = wo.astype(fp8)
        ohj[core, p, chunk * WJ + (jo - (gs % GJ) * WJ)] = fp8(1.0)
        wslab[core, p, chunk] = wo
    else:
        ESH = E // NCORES
        EPAD = CHUNKS * 128
        k = np.arange(EPAD)
        cc, p = k // 128, k % 128
        ip = np.zeros((NCORES, EPAD), dtype=np.int64)
        jp = np.zeros((NCORES, EPAD), dtype=np.int64)
        wp = np.zeros((NCORES, EPAD), dtype=np.float32)
        ip[:, :ESH] = i_idx.reshape(NCORES, ESH)
        jp[:, :ESH] = j_idx.reshape(NCORES, ESH)
        wp[:, :ESH] = w.reshape(NCORES, ESH)
        core = np.repeat(np.arange(NCORES), EPAD).reshape(NCORES, EPAD)
        pb = np.broadcast_to(p, (NCORES, EPAD))
        ohiw[core, pb, cc * 128 + ip] = wp.astype(fp8)
        ohj[core, pb, cc * 128 + jp] = fp8(1.0)
        wslab[core.reshape(-1), pb.reshape(-1), np.broadcast_to(cc, (NCORES, EPAD)).reshape(-1)] = 0  # noqa
        wslab[:] = 0
        wslab[core, pb, np.broadcast_to(cc, (NCORES, EPAD))] = wp

    # fuse into piece layout [IW_piece | J_piece | ...]
    oh = np.zeros((NCORES, 128, OH_W), dtype=fp8)
    off = 0
    for c0, c1 in PIECE_CHUNKS:
        niw = (c1 - c0) * WI
        nj = (c1 - c0) * WJ
        oh[:, :, off : off + niw] = ohiw[:, :, c0 * WI : c1 * WI]
        oh[:, :, off + niw : off + niw + nj] = ohj[:, :, c0 * WJ : c1 * WJ]
        off += niw + nj

    # pm: PT | w | Asc, byte layout
    asc = ((d == 1) * np.float32(-0.125)).astype(fp8)  # elementwise mask

    pm = np.zeros((NCORES, 128, PM_W), dtype=np.int8)
    pm[:, :, 0:PT_B] = PT.astype(fp8).view(np.int8)[None]
    pm[:, :, PT_B : PT_B + W_B] = np.ascontiguousarray(
        wslab.astype(bf16)
    ).view(np.int8)
    pm[:, :, PT_B + W_B : PT_B + W_B + 128] = asc.view(np.int8)[None]
    pm[:, :, PT_B + W_B + 128 : PT_B + W_B + 132] = (
        np.ones((128, 1), np.float32).view(np.int8)[None]
    )

    return [
        {
            "pm_in": np.ascontiguousarray(pm[i]),
            "oh_in": np.ascontiguousarray(oh[i]),
        }
        for i in range(NCORES)
    ]


def _combine(results):
    parts = np.stack([np.asarray(results[i]["out"]) for i in range(NCORES)])
    numer = float(parts[:, 0, 0].astype(np.float64).sum())
    wsum = float(parts[:, 1, 0].astype(np.float64).sum())
    return np.float32(numer / max(wsum, 1e-8))


def make_runner(nc, n_cores=NCORES):
    """jit-once mirror of bass2jax.run_bass_via_pjrt's multi-core branch so
    repeated kernel() calls reuse the compiled NEFF."""
    import jax
    import concourse.mybir as mybir
    from concourse.bass2jax import (
        Mesh,
        PartitionSpec,
        _bass_exec_p,
        install_neuronx_cc_hook,
        partition_id_tensor,
        shard_map,
    )

    install_neuronx_cc_hook()
    partition_name = nc.partition_id_tensor.name if nc.partition_id_tensor else None

    in_names, out_names, out_avals, zero_outs = [], [], [], []
    for alloc in nc.m.functions[0].allocations:
        if not isinstance(alloc, mybir.MemoryLocationSet):
            continue
        name = alloc.memorylocations[0].name
        if alloc.kind == "ExternalInput":
            if name != partition_name:
                in_names.append(name)
        elif alloc.kind == "ExternalOutput":
            shape = tuple(alloc.tensor_shape)
            dtype = mybir.dt.np(alloc.dtype)
            out_names.append(name)
            out_avals.append(jax.core.ShapedArray(shape, dtype))
            zero_outs.append(np.zeros(shape, dtype))
    n_params = len(in_names)
    n_outs = len(out_avals)
    all_names = in_names + out_names
    if partition_name is not None:
        all_names = all_names + [partition_name]
    donate = tuple(range(n_params, n_params + n_outs))

    def _body(*args):
        operands = list(args)
        if partition_name is not None:
            operands.append(partition_id_tensor())
        outs = _bass_exec_p.bind(
            *operands,
            out_avals=tuple(out_avals),
            in_names=tuple(all_names),
            out_names=tuple(out_names),
            lowering_input_output_aliases=(),
            sim_require_finite=True,
            sim_require_nnan=True,
            nc=nc,
        )
        return tuple(outs)

    devices = jax.devices()[:n_cores]
    mesh = Mesh(np.asarray(devices), ("core",))
    sharded = jax.jit(
        shard_map(
            _body,
            mesh=mesh,
            in_specs=(PartitionSpec("core"),) * (n_params + n_outs),
            out_specs=(PartitionSpec("core"),) * n_outs,
            check_rep=False,
        ),
        donate_argnums=donate,
        keep_unused=True,
    )

    def prep(in_maps):
        concat_in = [
            np.concatenate([np.asarray(m[name]) for m in in_maps], axis=0)
            for name in in_names
        ]
        return [jax.device_put(a) for a in concat_in]

    def run_dev(dev_in):
        concat_zeros = [
            np.zeros((n_cores * z.shape[0], *z.shape[1:]), z.dtype)
            for z in zero_outs
        ]
        out_arrs = sharded(*dev_in, *concat_zeros)
        out_arrs = [np.asarray(a) for a in out_arrs]
        return [
            {
                name: out_arrs[i].reshape(n_cores, *out_avals[i].shape)[c]
                for i, name in enumerate(out_names)
            }
            for c in range(n_cores)
        ]

    def run(in_maps):
        return run_dev(prep(in_maps))

    run.prep = prep
    run.run_dev = run_dev
    return run


_RUNNER = None


def kernel(P, d_hw, circuit_edge_pairs, circuit_edge_weights, _want_results=False):
    global _RUNNER
    in_maps = _shard_inputs(P, d_hw, circuit_edge_pairs, circuit_edge_weights)
    try:
        if _RUNNER is None:
            _RUNNER = make_runner(_get_built())
        results = _RUNNER(in_maps)
        res = None
    except Exception:
        if _want_results:
            raise
        # fallback: the stock SPMD runner (recompiles per call, but robust)
        from concourse.bass_utils import run_bass_kernel_spmd

        res = run_bass_kernel_spmd(
            _get_built(), in_maps, core_ids=list(range(NCORES))
        )
        results = res.results
    out = _combine(results)
    if _want_results:
        return out, res
    return out
